# revision 14
# baseline (speedup 1.0000x reference)
"""AttFusion (per-pixel single-query attention over ragged agent groups)
on 8 Trainium2 NeuronCores.

Problem: x (sum_cav=16, C=256, H=96, W=288) fp32, record_len (B=4,) int32.
For each scene b (agents x[off_b:off_b+L_b]) and each spatial location p:
  scores_l = <x[off_b], x[off_b+l]>_C / sqrt(C);  attn = softmax_l(scores)
  out[b,:,p] = sum_l attn_l * x[off_b+l,:,p]

Sharding: data-parallel over the spatial H axis (96 rows -> 12 rows/core;
each core handles all scenes for its spatial slice). The computation is
pointwise over pixels, so this is perfectly balanced with zero cross-core
communication, unlike group-parallel sharding (4 ragged groups / 8 cores).

Per-core layout: pixels-on-partitions ("transposed") for softmax+wsum:
- scores: qk_l = q (*) k_l elementwise in the NATURAL layout (DVE,
  fp32 in / bf16 out), then the PE reduces over C with tiny
  matmul(lhsT=qk-chunk, rhs=ones) ops that land scores directly as
  [128px, 1] columns in PSUM. This replaces the former scores-STT
  stream on the vector engine (measured -14 us wall): DVE tensor ops
  run at 1 elem/lane/cycle regardless of dtype on TRN2 hardware, so
  the PE reduction is the only way to take work off the DVE.
- softmax over agents: free-dim softmax on small (128, L) tiles; the
  1/sqrt(C) scale is folded into the Exp activation; no max-subtraction
  (scores*scale stays well inside fp32/exp range for this input).
- out += attn_l * v_l: fused scalar_tensor_tensor over transposed V
  chunks, attention weight as a per-partition scalar, ping-pong
  accumulators.
Transposes (c,p)<->(p,c) run on the tensor engine via identity matmul;
PSUM->SBUF moves and the output init/normalize run on the scalar engine.
Engine balance measured on HW: DVE ~276us busy, Scalar ~271, PE ~234,
DMA queues ~60% -- DVE and Scalar are co-critical at ~81% occupancy.
"""

import numpy as np
from contextlib import ExitStack

C = 256
H = 96
W = 288
N_CORES = 8
HS = H // N_CORES          # 12 rows per core
PS = HS * W                # 3456 pixels per core
CH = C // 128              # 2 c-halves
TP = 384                   # pixels per tile
NPT = PS // TP             # 9 tiles per scene
J2 = TP // 128             # chunks of 128 pixels per tile

_cache = {}


def _build(rec):
    import concourse.bacc as bacc
    import concourse.tile as tile
    from concourse import mybir
    from concourse.masks import make_identity

    rec = tuple(int(v) for v in rec)
    nb = len(rec)
    lmax = max(rec)
    offs = np.concatenate([[0], np.cumsum(rec)[:-1]]).tolist()
    total = int(sum(rec))
    f32 = mybir.dt.float32
    scale = float(1.0 / np.sqrt(C))
    Alu = mybir.AluOpType

    nc = bacc.Bacc("TRN2", target_bir_lowering=False, debug=False,
                   num_devices=N_CORES)
    x_ap = nc.dram_tensor("x", [total, C, HS, W], f32, kind="ExternalInput").ap()
    y_ap = nc.dram_tensor("y", [nb, C, HS, W], f32, kind="ExternalOutput").ap()
    # p-major dram views matching sbuf (partition, ch, pix) tiles
    xd = x_ap.rearrange("n (ch p) h w -> n p ch (h w)", ch=CH)
    yd = y_ap.rearrange("b (ch p) h w -> b p ch (h w)", ch=CH)

    with tile.TileContext(nc) as tc, ExitStack() as ctx:
        const_p = ctx.enter_context(tc.tile_pool(name="const", bufs=1))
        ident = const_p.tile([128, 128], f32)
        make_identity(nc, ident)
        ones = const_p.tile([128, 1], mybir.dt.bfloat16)
        nc.vector.memset(ones, 1.0)

        xnat_p = ctx.enter_context(tc.tile_pool(name="xnat", bufs=5))
        xT_p = ctx.enter_context(tc.tile_pool(name="xT", bufs=7))
        oacc_p = ctx.enter_context(tc.tile_pool(name="oacc", bufs=6))
        onat_p = ctx.enter_context(tc.tile_pool(name="onat", bufs=3))
        small_p = ctx.enter_context(tc.tile_pool(name="small", bufs=12))
        qk_p = ctx.enter_context(tc.tile_pool(name="qk", bufs=2))
        pxt_p = ctx.enter_context(tc.tile_pool(name="pxt", bufs=2, space="PSUM"))
        pob_p = ctx.enter_context(tc.tile_pool(name="pob", bufs=2, space="PSUM"))
        sT_p = ctx.enter_context(tc.tile_pool(name="sT", bufs=2, space="PSUM"))

        for b in range(nb):
            L = rec[b]
            off = offs[b]
            for pt in range(NPT):
                sl = slice(pt * TP, (pt + 1) * TP)
                xnat = xnat_p.tile([128, lmax, CH, TP], f32, tag="xnat")
                nc.sync.dma_start(out=xnat[:, :L, :, :],
                                  in_=xd[off:off + L, :, :, sl].rearrange(
                                      "n p ch x -> p n ch x"))
                # qk_l = q (*) k_l in natural layout (fp32 in, bf16 out);
                # the PE reduces over C below, replacing the scores STT stream
                qk = qk_p.tile([128, lmax, CH, TP], mybir.dt.bfloat16, tag="qk")
                for l in range(L):
                    nc.vector.tensor_tensor(out=qk[:, l], in0=xnat[:, 0],
                                            in1=xnat[:, l], op=Alu.mult)
                onat = onat_p.tile([128, CH, TP], f32, tag="onat")
                pob = pob_p.tile([128, J2, CH, 128], f32, tag="pob")
                for j in range(J2):
                    jsl = slice(j * 128, (j + 1) * 128)
                    # ---- transpose (c,p)->(p,c), 2 agents per psum bank ----
                    xT = xT_p.tile([128, lmax, CH * 128], f32, tag="xT")
                    for l0 in range(0, L, 2):
                        nl = min(2, L - l0)
                        pxt = pxt_p.tile([128, 2, CH * 128], f32, tag="pxt")
                        for dl in range(nl):
                            for ch in range(CH):
                                nc.tensor.transpose(
                                    pxt[:, dl, ch * 128:(ch + 1) * 128],
                                    xnat[:, l0 + dl, ch, jsl],
                                    ident)
                        nc.scalar.copy(out=xT[:, l0:l0 + nl, :],
                                       in_=pxt[:, :nl, :])
                    # ---- scores via PE: sT[:, l] = sum_c qk_l (transposed
                    # ones-reduce: lhsT=qk chunk, rhs=ones) ----
                    sT = sT_p.tile([128, lmax], f32, tag="sT")
                    for l in range(L):
                        for ch in range(CH):
                            nc.tensor.matmul(sT[:, l:l + 1],
                                             qk[:, l, ch, jsl], ones,
                                             start=(ch == 0),
                                             stop=(ch == CH - 1))
                    # ---- softmax over agents (free dim; scale folded into
                    # exp; no max-sub needed for this input distribution) ----
                    e = small_p.tile([128, lmax], f32, tag="e")
                    nc.scalar.activation(out=e[:, :L], in_=sT[:, :L],
                                         func=mybir.ActivationFunctionType.Exp,
                                         scale=scale)
                    z = small_p.tile([128, 1], f32, tag="z")
                    nc.vector.reduce_sum(out=z, in_=e[:, :L],
                                         axis=mybir.AxisListType.X)
                    r = small_p.tile([128, 1], f32, tag="r")
                    nc.vector.reciprocal(out=r, in_=z)
                    attn = small_p.tile([128, lmax], f32, tag="attn")
                    nc.scalar.activation(out=attn[:, :L], in_=e[:, :L],
                                         func=mybir.ActivationFunctionType.Copy,
                                         scale=r)
                    # ---- out_T = sum_l attn_l * v_l (per-partition scalars;
                    # ping-pong accumulators: in-place STT pays an RMW penalty) --
                    oacc = oacc_p.tile([128, 2, CH * 128], f32, tag="oacc")
                    nc.scalar.activation(out=oacc[:, 0, :], in_=xT[:, 0, :],
                                         func=mybir.ActivationFunctionType.Copy,
                                         scale=attn[:, 0:1])
                    for l in range(1, L):
                        nc.vector.scalar_tensor_tensor(
                            out=oacc[:, l % 2, :], in0=xT[:, l, :],
                            scalar=attn[:, l:l + 1], in1=oacc[:, (l - 1) % 2, :],
                            op0=Alu.mult, op1=Alu.add)
                    # ---- transpose back (p,c)->(c,p) ----
                    for ch in range(CH):
                        nc.tensor.transpose(pob[:, j, ch, :],
                                            oacc[:, (L - 1) % 2,
                                                 ch * 128:(ch + 1) * 128],
                                            ident)
                nc.scalar.copy(out=onat.rearrange("p ch (j x) -> p j ch x", j=J2),
                               in_=pob)
                nc.sync.dma_start(out=yd[b, :, :, sl], in_=onat)
    nc.compile()
    return nc


def _get_program(rec):
    key = tuple(int(v) for v in rec)
    if key not in _cache:
        _cache[key] = _build(key)
    return _cache[key]


def kernel(x, record_len):
    from concourse.bass_utils import run_bass_kernel_spmd

    x = np.ascontiguousarray(x, dtype=np.float32)
    rec = [int(v) for v in record_len]
    nb = len(rec)
    nc = _get_program(rec)
    in_maps = [
        {"x": np.ascontiguousarray(x[:, :, k * HS:(k + 1) * HS, :])}
        for k in range(N_CORES)
    ]
    res = run_bass_kernel_spmd(nc, in_maps, list(range(N_CORES)))
    out = np.empty((nb, C, H, W), dtype=np.float32)
    for k in range(N_CORES):
        out[:, :, k * HS:(k + 1) * HS, :] = res.results[k]["y"]
    return out



# revision 15
# speedup vs baseline: 1.0492x; 1.0492x over previous
"""AttFusion (per-pixel single-query attention over ragged agent groups)
on 8 Trainium2 NeuronCores.

Problem: x (sum_cav=16, C=256, H=96, W=288) fp32, record_len (B=4,) int32.
For each scene b (agents x[off_b:off_b+L_b]) and each spatial location p:
  scores_l = <x[off_b], x[off_b+l]>_C / sqrt(C);  attn = softmax_l(scores)
  out[b,:,p] = sum_l attn_l * x[off_b+l,:,p]

Sharding: data-parallel over the spatial H axis (96 rows -> 12 rows/core;
each core handles all scenes for its spatial slice). The computation is
pointwise over pixels, so this is perfectly balanced with zero cross-core
communication, unlike group-parallel sharding (4 ragged groups / 8 cores).

Per-core layout: pixels-on-partitions ("transposed") for softmax+wsum:
- scores: qk_l = q (*) k_l elementwise in the NATURAL layout (DVE,
  fp32 in / bf16 out), then the PE reduces over C with tiny
  matmul(lhsT=qk-chunk, rhs=ones) ops that land scores directly as
  [128px, 1] columns in PSUM. This replaces the former scores-STT
  stream on the vector engine (measured -14 us wall): DVE tensor ops
  run at 1 elem/lane/cycle regardless of dtype on TRN2 hardware, so
  the PE reduction is the only way to take work off the DVE.
- softmax over agents: free-dim softmax on small (128, L) tiles; the
  1/sqrt(C) scale is folded into the Exp activation; no max-subtraction
  (scores*scale stays well inside fp32/exp range for this input).
- out += attn_l * v_l: fused scalar_tensor_tensor over transposed V
  chunks, attention weight as a per-partition scalar, ping-pong
  accumulators.
Transposes (c,p)<->(p,c) run on the tensor engine via identity matmul;
PSUM->SBUF moves and the output init/normalize run on the scalar engine.
Engine balance measured on HW: DVE ~276us busy, Scalar ~271, PE ~234,
DMA queues ~60% -- DVE and Scalar are co-critical at ~81% occupancy.
"""

import numpy as np
from contextlib import ExitStack

C = 256
H = 96
W = 288
N_CORES = 8
HS = H // N_CORES          # 12 rows per core
PS = HS * W                # 3456 pixels per core
CH = C // 128              # 2 c-halves
TP = 384                   # pixels per tile
NPT = PS // TP             # 9 tiles per scene
J2 = TP // 128             # chunks of 128 pixels per tile

_cache = {}


def _build(rec):
    import concourse.bacc as bacc
    import concourse.tile as tile
    from concourse import mybir
    from concourse.masks import make_identity
    from concourse.bass import broadcast_tensor_aps

    rec = tuple(int(v) for v in rec)
    nb = len(rec)
    lmax = max(rec)
    offs = np.concatenate([[0], np.cumsum(rec)[:-1]]).tolist()
    total = int(sum(rec))
    f32 = mybir.dt.float32
    scale = float(1.0 / np.sqrt(C))
    Alu = mybir.AluOpType

    nc = bacc.Bacc("TRN2", target_bir_lowering=False, debug=False,
                   num_devices=N_CORES)
    x_ap = nc.dram_tensor("x", [total, C, HS, W], f32, kind="ExternalInput").ap()
    y_ap = nc.dram_tensor("y", [nb, C, HS, W], f32, kind="ExternalOutput").ap()
    # p-major dram views matching sbuf (partition, ch, pix) tiles
    xd = x_ap.rearrange("n (ch p) h w -> n p ch (h w)", ch=CH)
    yd = y_ap.rearrange("b (ch p) h w -> b p ch (h w)", ch=CH)

    with tile.TileContext(nc) as tc, ExitStack() as ctx:
        const_p = ctx.enter_context(tc.tile_pool(name="const", bufs=1))
        ident = const_p.tile([128, 128], f32)
        make_identity(nc, ident)
        ones = const_p.tile([128, 1], mybir.dt.bfloat16)
        nc.vector.memset(ones, 1.0)

        xnat_p = ctx.enter_context(tc.tile_pool(name="xnat", bufs=5))
        xT_p = ctx.enter_context(tc.tile_pool(name="xT", bufs=7))
        oacc_p = ctx.enter_context(tc.tile_pool(name="oacc", bufs=6))
        onat_p = ctx.enter_context(tc.tile_pool(name="onat", bufs=3))
        small_p = ctx.enter_context(tc.tile_pool(name="small", bufs=12))
        qk_p = ctx.enter_context(tc.tile_pool(name="qk", bufs=2))
        pxt_p = ctx.enter_context(tc.tile_pool(name="pxt", bufs=2, space="PSUM"))
        pob_p = ctx.enter_context(tc.tile_pool(name="pob", bufs=2, space="PSUM"))
        sT_p = ctx.enter_context(tc.tile_pool(name="sT", bufs=2, space="PSUM"))

        for b in range(nb):
            L = rec[b]
            off = offs[b]
            for pt in range(NPT):
                sl = slice(pt * TP, (pt + 1) * TP)
                xnat = xnat_p.tile([128, lmax, CH, TP], f32, tag="xnat")
                nc.sync.dma_start(out=xnat[:, :L, :, :],
                                  in_=xd[off:off + L, :, :, sl].rearrange(
                                      "n p ch x -> p n ch x"))
                # qk_l = q (*) k_l in natural layout (fp32 in, bf16 out);
                # the PE reduces over C below, replacing the scores STT stream
                qk = qk_p.tile([128, lmax, CH, TP], mybir.dt.bfloat16, tag="qk")
                l0 = 0
                while l0 < L:
                    n = min(4, L - l0)
                    if n == 1:
                        nc.vector.tensor_tensor(out=qk[:, l0], in0=xnat[:, 0],
                                                in1=xnat[:, l0], op=Alu.mult)
                    else:
                        in0b, in1b = broadcast_tensor_aps(
                            xnat[:, 0:1], xnat[:, l0:l0 + n])
                        nc.vector.tensor_tensor(out=qk[:, l0:l0 + n],
                                                in0=in0b, in1=in1b,
                                                op=Alu.mult)
                    l0 += n
                onat = onat_p.tile([128, CH, TP], f32, tag="onat")
                pob = pob_p.tile([128, J2, CH, 128], f32, tag="pob")
                for j in range(J2):
                    jsl = slice(j * 128, (j + 1) * 128)
                    # ---- transpose (c,p)->(p,c), 2 agents per psum bank ----
                    xT = xT_p.tile([128, lmax, CH * 128], f32, tag="xT")
                    for l0 in range(0, L, 2):
                        nl = min(2, L - l0)
                        pxt = pxt_p.tile([128, 2, CH * 128], f32, tag="pxt")
                        for dl in range(nl):
                            for ch in range(CH):
                                nc.tensor.transpose(
                                    pxt[:, dl, ch * 128:(ch + 1) * 128],
                                    xnat[:, l0 + dl, ch, jsl],
                                    ident)
                        nc.scalar.copy(out=xT[:, l0:l0 + nl, :],
                                       in_=pxt[:, :nl, :])
                    # ---- scores via PE: sT[:, l] = sum_c qk_l (transposed
                    # ones-reduce: lhsT=qk chunk, rhs=ones) ----
                    sT = sT_p.tile([128, lmax], f32, tag="sT")
                    for l in range(L):
                        for ch in range(CH):
                            nc.tensor.matmul(sT[:, l:l + 1],
                                             qk[:, l, ch, jsl], ones,
                                             start=(ch == 0),
                                             stop=(ch == CH - 1))
                    # ---- softmax over agents (free dim; scale folded into
                    # exp; no max-sub needed for this input distribution) ----
                    e = small_p.tile([128, lmax], f32, tag="e")
                    nc.scalar.activation(out=e[:, :L], in_=sT[:, :L],
                                         func=mybir.ActivationFunctionType.Exp,
                                         scale=scale)
                    z = small_p.tile([128, 1], f32, tag="z")
                    nc.vector.reduce_sum(out=z, in_=e[:, :L],
                                         axis=mybir.AxisListType.X)
                    r = small_p.tile([128, 1], f32, tag="r")
                    nc.vector.reciprocal(out=r, in_=z)
                    attn = small_p.tile([128, lmax], f32, tag="attn")
                    nc.vector.tensor_scalar(out=attn[:, :L], in0=e[:, :L],
                                            scalar1=r, scalar2=None,
                                            op0=Alu.mult)
                    # ---- out_T = sum_l attn_l * v_l (per-partition scalars;
                    # ping-pong accumulators: in-place STT pays an RMW penalty) --
                    oacc = oacc_p.tile([128, 2, CH * 128], f32, tag="oacc")
                    nc.scalar.activation(out=oacc[:, 0, :], in_=xT[:, 0, :],
                                         func=mybir.ActivationFunctionType.Copy,
                                         scale=attn[:, 0:1])
                    for l in range(1, L):
                        nc.vector.scalar_tensor_tensor(
                            out=oacc[:, l % 2, :], in0=xT[:, l, :],
                            scalar=attn[:, l:l + 1], in1=oacc[:, (l - 1) % 2, :],
                            op0=Alu.mult, op1=Alu.add)
                    # ---- transpose back (p,c)->(c,p) ----
                    for ch in range(CH):
                        nc.tensor.transpose(pob[:, j, ch, :],
                                            oacc[:, (L - 1) % 2,
                                                 ch * 128:(ch + 1) * 128],
                                            ident)
                nc.scalar.copy(out=onat.rearrange("p ch (j x) -> p j ch x", j=J2),
                               in_=pob)
                nc.sync.dma_start(out=yd[b, :, :, sl], in_=onat)
    nc.compile()
    return nc


def _get_program(rec):
    key = tuple(int(v) for v in rec)
    if key not in _cache:
        _cache[key] = _build(key)
    return _cache[key]


def kernel(x, record_len):
    from concourse.bass_utils import run_bass_kernel_spmd

    x = np.ascontiguousarray(x, dtype=np.float32)
    rec = [int(v) for v in record_len]
    nb = len(rec)
    nc = _get_program(rec)
    in_maps = [
        {"x": np.ascontiguousarray(x[:, :, k * HS:(k + 1) * HS, :])}
        for k in range(N_CORES)
    ]
    res = run_bass_kernel_spmd(nc, in_maps, list(range(N_CORES)))
    out = np.empty((nb, C, H, W), dtype=np.float32)
    for k in range(N_CORES):
        out[:, :, k * HS:(k + 1) * HS, :] = res.results[k]["y"]
    return out

